# revision 35
# baseline (speedup 1.0000x reference)
"""Multi-head attention layer (B=2,S=2048,D=1024,H=16) on 8 TRN2 NeuronCores.

Sharding: data parallel over batch (2) x tensor parallel over heads (4 heads
per core).  Each core computes, for its (batch b, head-group hg):
  QT = (X_b @ Wq[:,cols] + bq + emotion)^T         [256, S]     (n on partitions)
  KT = (Xv_b @ Wk[:,cols] + bk)^T                  [256, Skv]   (compacted keys)
  V  = Xv_b @ Wv[:,cols] + bv                      [Skv, 256]   (natural, +ones col)
  scoresT[j,i] = KT_h-slices x QT_h, exp fused with 1/8 scale + key mask bias,
  OT_h = V_h_aug^T @ PT  (row 64 = softmax denominator l via the ones column),
  out_partial = (OT/l)^T @ Wo[rows,:]              [S, D]
Host compacts the key/value positions by the attention mask, then sums the 4
partial outputs per batch and adds bo.

Matmul operands are bf16 (fp32r draws enough PE power to trip the HAM
duty-cycle throttle); PSUM accumulation stays fp32.  Phase 2 is paced by the
scalar-engine exp; the PE is kept streaming by interleaving the NEXT query
block's Q projection and the PREVIOUS block's output projection as fine
pending units, one per j iteration.
"""
import math
import sys

sys.path.insert(0, "/opt/trn_rl_repo")

import ml_dtypes
import numpy as np

import concourse.bass as bass
import concourse.tile as tile
from concourse import bacc, mybir
from concourse.bass_utils import run_bass_kernel_spmd

B, S, D, H = 2, 2048, 1024, 16
DH = D // H          # 64
HPC = 4              # heads per core
NCOL = HPC * DH      # 256 columns of Wq/Wk/Wv per core
NC2 = NCOL // 128    # 2 partition-chunks of the head dim
ND = D // 128        # 8 contraction chunks
NI = S // 512        # 4 query 512-chunks
F32 = mybir.dt.float32
F32R = mybir.dt.float32r
BF16 = mybir.dt.bfloat16
BF16_NP = ml_dtypes.bfloat16
AF = mybir.ActivationFunctionType

_PROGRAM_CACHE = {}


def _chunks(total, step):
    out = []
    o = 0
    while o < total:
        out.append((o, min(step, total - o)))
        o += step
    return out


def build_program(skv: int, debug: bool = False):
    """One NeuronCore's program; SPMD across 8 cores with different data."""
    nj = skv // 128
    kchunks = _chunks(skv, 512)
    nc = bacc.Bacc("TRN2", target_bir_lowering=False, debug=debug, num_devices=8)

    xt = nc.declare_dram_parameter("xt", [D, S], BF16, isOutput=False)
    xtkv = nc.declare_dram_parameter("xtkv", [D, skv], BF16, isOutput=False)
    wq = nc.declare_dram_parameter("wq", [D, NCOL], BF16, isOutput=False)
    wk = nc.declare_dram_parameter("wk", [D, NCOL], BF16, isOutput=False)
    wv = nc.declare_dram_parameter("wv", [D, NCOL], BF16, isOutput=False)
    wo = nc.declare_dram_parameter("wo", [NCOL, D], BF16, isOutput=False)
    bq = nc.declare_dram_parameter("bq", [NCOL], F32, isOutput=False)
    bk = nc.declare_dram_parameter("bk", [NCOL], F32, isOutput=False)
    bv = nc.declare_dram_parameter("bv", [NCOL], F32, isOutput=False)
    ew = nc.declare_dram_parameter("ew", [NCOL], F32, isOutput=False)
    maskb = nc.declare_dram_parameter("maskb", [skv], F32, isOutput=False)
    out = nc.declare_dram_parameter("out", [S, D], F32, isOutput=True)

    with tile.TileContext(nc) as tc:
        with tc.tile_pool(name="singles", bufs=1) as singles:
            # --- persistent SBUF tiles -----------------------------------
            twqa = singles.tile([128, ND * NCOL], BF16, tag="wqa", name="twqa")
            twka = singles.tile([128, ND * NCOL], BF16, tag="wka", name="twka")
            twva = singles.tile([128, ND * NCOL], BF16, tag="wva", name="twva")
            twoa = singles.tile([128, NC2 * D], BF16, tag="woa", name="twoa")
            txkva = singles.tile([128, ND * skv], BF16, tag="xkva", name="txkva")
            txta = singles.tile([128, ND * S], BF16, tag="xta", name="txta")
            twq = [twqa[:, d * NCOL:(d + 1) * NCOL] for d in range(ND)]
            twk = [twka[:, d * NCOL:(d + 1) * NCOL] for d in range(ND)]
            twv = [twva[:, d * NCOL:(d + 1) * NCOL] for d in range(ND)]
            two = [twoa[:, c * D:(c + 1) * D] for c in range(NC2)]
            txkv = [txkva[:, d * skv:(d + 1) * skv] for d in range(ND)]
            # xt chunk for (d, i): [128, 512]
            txt = [
                [txta[:, (d * NI + i) * 512:(d * NI + i + 1) * 512] for i in range(NI)]
                for d in range(ND)
            ]
            tqt = [singles.tile([128, S], BF16, tag=f"qt{c}", name=f"qt{c}") for c in range(NC2)]
            tkt = [singles.tile([128, skv], BF16, tag=f"kt{c}", name=f"kt{c}") for c in range(NC2)]
            tv = [
                [singles.tile([128, DH + 1], BF16, tag=f"v{h}_{j}", name=f"v{h}_{j}") for j in range(nj)]
                for h in range(HPC)
            ]
            tot = [singles.tile([128, S], F32, tag=f"ot{c}", name=f"ot{c}") for c in range(NC2)]
            totn = [singles.tile([128, S], BF16, tag=f"otn{c}", name=f"otn{c}") for c in range(NC2)]
            # softmax denominators: rows 0/32/64/96 hold heads 0..3; the
            # custom-DVE reciprocal only works on full-width offset-0 APs,
            # so stage the rows and batch it
            tstage = singles.tile([97, S], F32, tag="lstage", name="tstage")
            trecf = singles.tile([97, S], F32, tag="lrecf", name="trecf")
            trecb = singles.tile([97, S], BF16, tag="lrecb", name="trecb")
            tones4 = singles.tile([97, 64], BF16, tag="ones4", name="tones4")
            tonesf = singles.tile([128, 64], F32, tag="onesf", name="tonesf")
            tmb = [singles.tile([128, 1], F32, tag=f"mb{j}", name=f"mb{j}") for j in range(nj)]
            tbiasq = [singles.tile([128, 1], F32, tag=f"bq{c}", name=f"bq{c}") for c in range(NC2)]
            tbiask = [singles.tile([128, 1], F32, tag=f"bk{c}", name=f"bkt{c}") for c in range(NC2)]
            tbq_raw = [singles.tile([128, 1], F32, tag=f"bqr{c}", name=f"bqr{c}") for c in range(NC2)]
            tew_raw = [singles.tile([128, 1], F32, tag=f"ewr{c}", name=f"ewr{c}") for c in range(NC2)]
            tbvb = singles.tile([128, NCOL], F32, tag="bvb", name="bvb")

            # --- input DMAs ----------------------------------------------
            # sync queue order = critical path order: wk+xtkv first (K proj
            # streams per-d), wv (V proj), wq + xt[i=0] (Q proj of block 0),
            # xt[i=1], wo (needed at block 1), xt[i=2,3].
            for d in range(ND):
                nc.sync.dma_start(out=twk[d], in_=wk[d * 128:(d + 1) * 128, :])
            for d in range(ND):
                nc.sync.dma_start(out=txkv[d], in_=xtkv[d * 128:(d + 1) * 128, :])
            for d in range(ND):
                nc.sync.dma_start(out=twq[d], in_=wq[d * 128:(d + 1) * 128, :])
            for d in range(ND):
                nc.sync.dma_start(
                    out=txt[d][0],
                    in_=xt[d * 128:(d + 1) * 128, 0:512],
                )
            for d in range(ND):
                nc.sync.dma_start(out=twv[d], in_=wv[d * 128:(d + 1) * 128, :])
            for d in range(ND):
                nc.sync.dma_start(
                    out=txt[d][1],
                    in_=xt[d * 128:(d + 1) * 128, 512:1024],
                )
            for c in range(NC2):
                nc.sync.dma_start(out=two[c], in_=wo[c * 128:(c + 1) * 128, :])
            for i in (2, 3):
                for d in range(ND):
                    nc.sync.dma_start(
                        out=txt[d][i],
                        in_=xt[d * 128:(d + 1) * 128, i * 512:(i + 1) * 512],
                    )
            # small tiles on the gpsimd (software DGE) queue
            for c in range(NC2):
                nc.gpsimd.dma_start(
                    out=tbq_raw[c], in_=bq[c * 128:(c + 1) * 128].unsqueeze(1)
                )
                nc.gpsimd.dma_start(
                    out=tew_raw[c], in_=ew[c * 128:(c + 1) * 128].unsqueeze(1)
                )
                nc.gpsimd.dma_start(
                    out=tbiask[c], in_=bk[c * 128:(c + 1) * 128].unsqueeze(1)
                )
                nc.vector.tensor_add(out=tbiasq[c], in0=tbq_raw[c], in1=tew_raw[c])
            for j in range(nj):
                nc.gpsimd.dma_start(
                    out=tmb[j], in_=maskb[j * 128:(j + 1) * 128].unsqueeze(1)
                )
            bvap = bv.ap()
            bv_bcast = bass.AP(
                tensor=bvap.tensor, offset=bvap.offset, ap=[[0, 128]] + list(bvap.ap)
            )
            nc.gpsimd.dma_start(out=tbvb, in_=bv_bcast)
            # memset cannot write float32r, so round ones through the DVE.
            nc.vector.memset(tonesf, 1.0)
            # rows of tstage between the 32h anchors are swept by the batched
            # reciprocal; init once so they stay finite
            nc.vector.memset(tstage, 1.0)
            for h in range(HPC):
                nc.vector.tensor_copy(
                    out=tones4[32 * h:32 * h + 1, :], in_=tonesf[0:1, :]
                )

            # --- phase 1: K, V projections + Q projection of block 0 -----
            # ones column of V (gpsimd copy keeps the DVE free)
            for h in range(HPC):
                for j in range(nj):
                    nc.gpsimd.tensor_copy(
                        out=tv[h][j][:, DH:DH + 1], in_=tonesf[:, 0:1]
                    )
            # KT: d-outer so each txkv[d] chunk is consumed as it lands
            with tc.tile_pool(name="pk", bufs=1, space="PSUM") as pk:
                kps = [
                    [pk.tile([128, 512], F32, tag=f"pk{ci}_{c}", name="pkt") for c in range(NC2)]
                    for ci in range(len(kchunks))
                ]
                for d in range(ND):
                    for ci, (jo, jw) in enumerate(kchunks):
                        for c in range(NC2):
                            nc.tensor.matmul(
                                kps[ci][c][:, 0:jw],
                                twk[d][:, c * 128:(c + 1) * 128],
                                txkv[d][:, jo:jo + jw],
                                start=(d == 0),
                                stop=(d == ND - 1),
                            )
                for ci, (jo, jw) in enumerate(kchunks):
                    for c in range(NC2):
                        nc.vector.tensor_scalar_add(
                            out=tkt[c][:, jo:jo + jw],
                            in0=kps[ci][c][:, 0:jw],
                            scalar1=tbiask[c],
                        )
            with (
                tc.tile_pool(name="pproj", bufs=2, space="PSUM") as pproj,
                tc.tile_pool(name="pv", bufs=2, space="PSUM") as pv,
            ):
                # Q projection of query block 0 (blocks 1..3 are interleaved
                # into phase 2 as pending units)
                for c in range(NC2):
                    ps = pproj.tile([128, 512], F32, tag="pp", name="ppt")
                    for d in range(ND):
                        nc.tensor.matmul(
                            ps,
                            twq[d][:, c * 128:(c + 1) * 128],
                            txt[d][0],
                            start=(d == 0),
                            stop=(d == ND - 1),
                        )
                    nc.vector.tensor_scalar_add(
                        out=tqt[c][:, 0:512], in0=ps, scalar1=tbiasq[c]
                    )
                # V[0] only; V[1..nj-1] stream as priority units inside the
                # first c-block's j-loop so phase 2 starts right after Q0
                ps = pv.tile([128, NCOL], F32, tag="pv", name="pvt")
                for d in range(ND):
                    nc.tensor.matmul(
                        ps,
                        txkv[d][:, 0:128],
                        twv[d],
                        start=(d == 0),
                        stop=(d == ND - 1),
                    )
                for h in range(HPC):
                    nc.vector.tensor_add(
                        out=tv[h][0][:, 0:DH],
                        in0=ps[:, h * DH:(h + 1) * DH],
                        in1=tbvb[:, h * DH:(h + 1) * DH],
                    )

            # --- phase 2: attention + normalize + output projection ------
            # The j-loop is paced by the ACT-engine exp; one pending unit
            # (~0.9us of independent PE work) is popped per j iteration so
            # the PE queue never drains: output projection of block i-1 and
            # Q projection of block i+1.
            with (
                tc.tile_pool(name="pts", bufs=3) as pts,
                tc.tile_pool(name="obuf", bufs=4) as obuf,
                tc.tile_pool(name="ps2", bufs=2, space="PSUM") as ps2,
                tc.tile_pool(name="pot", bufs=2, space="PSUM") as pot,
                tc.tile_pool(name="plf", bufs=2, space="PSUM") as plf,
            ):
                pending = []

                def emit_pf(i, so, n):
                    # one outproj unit: 2 accumulating matmuls -> copy -> DMA
                    sidx = i * 4 + so
                    ssl = slice(sidx * 128, (sidx + 1) * 128)
                    nsl = slice(n * 512, (n + 1) * 512)
                    pf = plf.tile([128, 512], F32, tag="plf", name="pft")
                    for c in range(NC2):
                        nc.tensor.matmul(
                            pf,
                            totn[c][:, ssl],
                            two[c][:, nsl],
                            start=(c == 0),
                            stop=(c == NC2 - 1),
                        )
                    ob = obuf.tile([128, 512], F32, tag="ob", name="obt")
                    nc.vector.tensor_copy(out=ob, in_=pf)
                    nc.sync.dma_start(out=out[ssl, nsl], in_=ob)

                def emit_norm(i):
                    # batched reciprocal + bf16 cast of 1/l, then per-head
                    # ones-row broadcast + normalize (deferred off the block
                    # boundary into later j-loop pop slots)
                    isl = slice(i * 512, (i + 1) * 512)
                    nc.vector.reciprocal_approx_fast(
                        out=trecf[:, isl], in_=tstage[:, isl]
                    )
                    nc.vector.tensor_copy(out=trecb[:, isl], in_=trecf[:, isl])
                    for c in range(NC2):
                        hA, hB = 2 * c, 2 * c + 1
                        plA = plf.tile([64, 512], F32, tag="plf", name="plA")
                        plB = plf.tile([64, 512], F32, tag="plf", name="plB")
                        nc.tensor.matmul(
                            plA,
                            tones4[32 * hA:32 * hA + 1, :],
                            trecb[32 * hA:32 * hA + 1, isl],
                            start=True,
                            stop=True,
                            tile_position=(32 * hA, 0),
                        )
                        nc.tensor.matmul(
                            plB,
                            tones4[32 * hB:32 * hB + 1, :],
                            trecb[32 * hB:32 * hB + 1, isl],
                            start=True,
                            stop=True,
                            tile_position=(32 * hB, 0),
                        )
                        nc.vector.tensor_mul(
                            out=totn[c][0:64, isl], in0=tot[c][0:64, isl], in1=plA
                        )
                        nc.vector.tensor_mul(
                            out=totn[c][64:128, isl], in0=tot[c][64:128, isl], in1=plB
                        )

                def emit_v(j):
                    # V projection of key block j into a borrowed ps2 tile
                    ps = ps2.tile([128, 1024], F32, tag="ps", name="pvt2")
                    for d in range(ND):
                        nc.tensor.matmul(
                            ps[:, 0:NCOL],
                            txkv[d][:, j * 128:(j + 1) * 128],
                            twv[d],
                            start=(d == 0),
                            stop=(d == ND - 1),
                        )
                    for h in range(HPC):
                        nc.vector.tensor_add(
                            out=tv[h][j][:, 0:DH],
                            in0=ps[:, h * DH:(h + 1) * DH],
                            in1=tbvb[:, h * DH:(h + 1) * DH],
                        )

                v_units = [lambda j=j: emit_v(j) for j in range(1, nj)]

                def emit_q(i, c):
                    # one Q-proj unit: 8 accumulating matmuls (borrowing a
                    # ps2 tile; scores j+1 uses the other buffer) + bias add
                    isl = slice(i * 512, (i + 1) * 512)
                    ps = ps2.tile([128, 1024], F32, tag="ps", name="pqt")
                    for d in range(ND):
                        nc.tensor.matmul(
                            ps[:, 0:512],
                            twq[d][:, c * 128:(c + 1) * 128],
                            txt[d][i],
                            start=(d == 0),
                            stop=(d == ND - 1),
                        )
                    nc.vector.tensor_scalar_add(
                        out=tqt[c][:, isl], in0=ps[:, 0:512], scalar1=tbiasq[c]
                    )

                for i in range(NI):
                    isl = slice(i * 512, (i + 1) * 512)
                    if i + 1 < NI:
                        for c in range(NC2):
                            pending.append((i + 1, lambda i=i, c=c: emit_q(i + 1, c)))
                    # force-emit units whose results this block depends on
                    # (Q projection of block i) if the queue hasn't drained
                    while any(dl <= i for dl, _ in pending):
                        pending.pop(0)[1]()
                    for c in range(NC2):
                        hA, hB = 2 * c, 2 * c + 1
                        potA = pot.tile([DH + 1, 512], F32, tag="pot", name="pott")
                        potB = pot.tile([DH + 1, 512], F32, tag="pot", name="pott")
                        pts_hist = []
                        for j in range(nj):
                            pscore = ps2.tile([128, 1024], F32, tag="ps", name="pscore")
                            nc.tensor.matmul(
                                pscore[:, 0:512],
                                tkt[c][0:64, j * 128:(j + 1) * 128],
                                tqt[c][0:64, isl],
                                start=True,
                                stop=True,
                                tile_position=(0, 0),
                            )
                            nc.tensor.matmul(
                                pscore[:, 512:1024],
                                tkt[c][64:128, j * 128:(j + 1) * 128],
                                tqt[c][64:128, isl],
                                start=True,
                                stop=True,
                                tile_position=(64, 0),
                            )
                            pt = pts.tile([128, 1024], BF16, tag="pt", name="ptile")
                            nc.scalar.activation(
                                out=pt,
                                in_=pscore,
                                func=AF.Exp,
                                bias=tmb[j],
                                scale=1.0 / math.sqrt(DH),
                            )
                            pts_hist.append(pt)
                            if j > 0:
                                pprev = pts_hist[j - 1]
                                nc.tensor.matmul(
                                    potA, tv[hA][j - 1], pprev[:, 0:512],
                                    start=(j - 1 == 0), stop=False,
                                )
                                nc.tensor.matmul(
                                    potB, tv[hB][j - 1], pprev[:, 512:1024],
                                    start=(j - 1 == 0), stop=False,
                                )
                            if v_units:
                                v_units.pop(0)()
                            elif pending:
                                pending.pop(0)[1]()
                        nc.tensor.matmul(
                            potA, tv[hA][nj - 1], pts_hist[nj - 1][:, 0:512],
                            start=(nj == 1), stop=True,
                        )
                        nc.tensor.matmul(
                            potB, tv[hB][nj - 1], pts_hist[nj - 1][:, 512:1024],
                            start=(nj == 1), stop=True,
                        )
                        nc.vector.tensor_copy(out=tot[c][0:64, isl], in_=potA[0:DH, :])
                        nc.vector.tensor_copy(out=tot[c][64:128, isl], in_=potB[0:DH, :])
                        nc.vector.tensor_copy(
                            out=tstage[32 * hA:32 * hA + 1, isl],
                            in_=potA[DH:DH + 1, :],
                        )
                        nc.vector.tensor_copy(
                            out=tstage[32 * hB:32 * hB + 1, isl],
                            in_=potB[DH:DH + 1, :],
                        )
                    pending.append((NI + 1, lambda i=i: emit_norm(i)))
                    for so in range(4):
                        for n in range(2):
                            pending.append(
                                (NI + 1, lambda i=i, so=so, n=n: emit_pf(i, so, n))
                            )
                while pending:
                    pending.pop(0)[1]()

    nc.compile()
    return nc


def _get_program(skv):
    if skv not in _PROGRAM_CACHE:
        _PROGRAM_CACHE[skv] = build_program(skv)
    return _PROGRAM_CACHE[skv]


def _shard_inputs(hidden_states, attention_mask, Wq, bq, Wk, bk, Wv, bv,
                  emotion_w, Wo, bo):
    hs = np.asarray(hidden_states, dtype=np.float32)
    mask = np.asarray(attention_mask)
    Wq = np.asarray(Wq, dtype=np.float32)
    Wk = np.asarray(Wk, dtype=np.float32)
    Wv = np.asarray(Wv, dtype=np.float32)
    Wo = np.asarray(Wo, dtype=np.float32)
    bq = np.asarray(bq, dtype=np.float32)
    bk = np.asarray(bk, dtype=np.float32)
    bv = np.asarray(bv, dtype=np.float32)
    ew = np.asarray(emotion_w, dtype=np.float32)

    idx = [np.nonzero(mask[b])[0] for b in range(B)]
    sv = max(len(ix) for ix in idx)
    skv = max(128, ((sv + 127) // 128) * 128)

    in_maps = []
    for b in range(B):
        xt_b = np.ascontiguousarray(hs[b].T.astype(BF16_NP))  # [D, S]
        xtkv_b = np.zeros((D, skv), dtype=BF16_NP)
        xtkv_b[:, : len(idx[b])] = hs[b][idx[b]].T.astype(BF16_NP)
        maskb_b = np.zeros(skv, dtype=np.float32)
        maskb_b[len(idx[b]):] = -1e30
        for hg in range(H // HPC):
            cols = slice(hg * NCOL, (hg + 1) * NCOL)
            in_maps.append(
                {
                    "xt": xt_b,
                    "xtkv": xtkv_b,
                    "wq": np.ascontiguousarray(Wq[:, cols].astype(BF16_NP)),
                    "wk": np.ascontiguousarray(Wk[:, cols].astype(BF16_NP)),
                    "wv": np.ascontiguousarray(Wv[:, cols].astype(BF16_NP)),
                    "wo": np.ascontiguousarray(Wo[cols, :].astype(BF16_NP)),
                    "bq": np.ascontiguousarray(bq[cols]),
                    "bk": np.ascontiguousarray(bk[cols]),
                    "bv": np.ascontiguousarray(bv[cols]),
                    "ew": np.ascontiguousarray(
                        ew[hg * HPC:(hg + 1) * HPC].reshape(NCOL)
                    ),
                    "maskb": maskb_b,
                }
            )
    return in_maps, skv, np.asarray(bo, dtype=np.float32)


def run(inputs, trace=False, trace_kwargs=None):
    in_maps, skv, bo = _shard_inputs(**inputs)
    nc = _get_program(skv)
    res = run_bass_kernel_spmd(
        nc,
        in_maps,
        core_ids=list(range(8)),
        trace=trace,
        **(trace_kwargs or {}),
    )
    out = np.zeros((B, S, D), dtype=np.float32)
    for b in range(B):
        acc = np.zeros((S, D), dtype=np.float64)
        for hg in range(4):
            acc += res.results[b * 4 + hg]["out"]
        out[b] = (acc + bo).astype(np.float32)
    return out, res


def kernel(**inputs):
    out, _ = run(inputs, trace=False)
    return out


# revision 37
# speedup vs baseline: 1.0294x; 1.0294x over previous
"""Multi-head attention layer (B=2,S=2048,D=1024,H=16) on 8 TRN2 NeuronCores.

Sharding: data parallel over batch (2) x tensor parallel over heads (4 heads
per core).  Each core computes, for its (batch b, head-group hg):
  QT = (X_b @ Wq[:,cols] + bq + emotion)^T         [256, S]     (n on partitions)
  KT = (Xv_b @ Wk[:,cols] + bk)^T                  [256, Skv]   (compacted keys)
  V  = Xv_b @ Wv[:,cols] + bv                      [Skv, 256]   (natural, +ones col)
  scoresT[j,i] = KT_h-slices x QT_h, exp fused with 1/8 scale + key mask bias,
  OT_h = V_h_aug^T @ PT  (row 64 = softmax denominator l via the ones column),
  out_partial = (OT/l)^T @ Wo[rows,:]              [S, D]
Host compacts the key/value positions by the attention mask, then sums the 4
partial outputs per batch and adds bo.

Matmul operands are bf16 (fp32r draws enough PE power to trip the HAM
duty-cycle throttle); PSUM accumulation stays fp32.  Phase 2 is paced by the
scalar-engine exp; the PE is kept streaming by interleaving the NEXT query
block's Q projection and the PREVIOUS block's output projection as fine
pending units, one per j iteration.
"""
import math
import sys

sys.path.insert(0, "/opt/trn_rl_repo")

import ml_dtypes
import numpy as np

import concourse.bass as bass
import concourse.tile as tile
from concourse import bacc, mybir
from concourse.bass_utils import run_bass_kernel_spmd

B, S, D, H = 2, 2048, 1024, 16
DH = D // H          # 64
HPC = 4              # heads per core
NCOL = HPC * DH      # 256 columns of Wq/Wk/Wv per core
NC2 = NCOL // 128    # 2 partition-chunks of the head dim
ND = D // 128        # 8 contraction chunks
NI = S // 512        # 4 query 512-chunks
F32 = mybir.dt.float32
F32R = mybir.dt.float32r
BF16 = mybir.dt.bfloat16
BF16_NP = ml_dtypes.bfloat16
AF = mybir.ActivationFunctionType

_PROGRAM_CACHE = {}


def _chunks(total, step):
    out = []
    o = 0
    while o < total:
        out.append((o, min(step, total - o)))
        o += step
    return out


def build_program(skv: int, debug: bool = False):
    """One NeuronCore's program; SPMD across 8 cores with different data."""
    nj = skv // 128
    kchunks = _chunks(skv, 512)
    nc = bacc.Bacc("TRN2", target_bir_lowering=False, debug=debug, num_devices=8)

    xt = nc.declare_dram_parameter("xt", [D, S], BF16, isOutput=False)
    xtkv = nc.declare_dram_parameter("xtkv", [D, skv], BF16, isOutput=False)
    wq = nc.declare_dram_parameter("wq", [D, NCOL], BF16, isOutput=False)
    wk = nc.declare_dram_parameter("wk", [D, NCOL], BF16, isOutput=False)
    wv = nc.declare_dram_parameter("wv", [D, NCOL], BF16, isOutput=False)
    wo = nc.declare_dram_parameter("wo", [NCOL, D], BF16, isOutput=False)
    bq = nc.declare_dram_parameter("bq", [NCOL], F32, isOutput=False)
    bk = nc.declare_dram_parameter("bk", [NCOL], F32, isOutput=False)
    bv = nc.declare_dram_parameter("bv", [NCOL], F32, isOutput=False)
    ew = nc.declare_dram_parameter("ew", [NCOL], F32, isOutput=False)
    maskb = nc.declare_dram_parameter("maskb", [skv], F32, isOutput=False)
    out = nc.declare_dram_parameter("out", [S, D], F32, isOutput=True)

    with tile.TileContext(nc) as tc:
        with tc.tile_pool(name="singles", bufs=1) as singles:
            # --- persistent SBUF tiles -----------------------------------
            twqa = singles.tile([128, ND * NCOL], BF16, tag="wqa", name="twqa")
            twka = singles.tile([128, ND * NCOL], BF16, tag="wka", name="twka")
            twva = singles.tile([128, ND * NCOL], BF16, tag="wva", name="twva")
            twoa = singles.tile([128, NC2 * D], BF16, tag="woa", name="twoa")
            txkva = singles.tile([128, ND * skv], BF16, tag="xkva", name="txkva")
            txta = singles.tile([128, ND * S], BF16, tag="xta", name="txta")
            twq = [twqa[:, d * NCOL:(d + 1) * NCOL] for d in range(ND)]
            twk = [twka[:, d * NCOL:(d + 1) * NCOL] for d in range(ND)]
            twv = [twva[:, d * NCOL:(d + 1) * NCOL] for d in range(ND)]
            two = [twoa[:, c * D:(c + 1) * D] for c in range(NC2)]
            txkv = [txkva[:, d * skv:(d + 1) * skv] for d in range(ND)]
            # xt chunk for (d, i): [128, 512]
            txt = [
                [txta[:, (d * NI + i) * 512:(d * NI + i + 1) * 512] for i in range(NI)]
                for d in range(ND)
            ]
            tqt = [singles.tile([128, S], BF16, tag=f"qt{c}", name=f"qt{c}") for c in range(NC2)]
            tkt = [singles.tile([128, skv], BF16, tag=f"kt{c}", name=f"kt{c}") for c in range(NC2)]
            tv = [
                [singles.tile([128, DH + 1], BF16, tag=f"v{h}_{j}", name=f"v{h}_{j}") for j in range(nj)]
                for h in range(HPC)
            ]
            tot = [singles.tile([128, S], F32, tag=f"ot{c}", name=f"ot{c}") for c in range(NC2)]
            totn = [singles.tile([128, S], BF16, tag=f"otn{c}", name=f"otn{c}") for c in range(NC2)]
            # softmax denominators: rows 0/32/64/96 hold heads 0..3; the
            # custom-DVE reciprocal only works on full-width offset-0 APs,
            # so stage the rows and batch it
            tstage = singles.tile([97, S], F32, tag="lstage", name="tstage")
            trecf = singles.tile([97, S], F32, tag="lrecf", name="trecf")
            trecb = singles.tile([97, S], BF16, tag="lrecb", name="trecb")
            tones4 = singles.tile([97, 64], BF16, tag="ones4", name="tones4")
            tonesf = singles.tile([128, 64], F32, tag="onesf", name="tonesf")
            tmb = [singles.tile([128, 1], F32, tag=f"mb{j}", name=f"mb{j}") for j in range(nj)]
            tbiasq = [singles.tile([128, 1], F32, tag=f"bq{c}", name=f"bq{c}") for c in range(NC2)]
            tbiask = [singles.tile([128, 1], F32, tag=f"bk{c}", name=f"bkt{c}") for c in range(NC2)]
            tbq_raw = [singles.tile([128, 1], F32, tag=f"bqr{c}", name=f"bqr{c}") for c in range(NC2)]
            tew_raw = [singles.tile([128, 1], F32, tag=f"ewr{c}", name=f"ewr{c}") for c in range(NC2)]
            tbvb = singles.tile([128, NCOL], F32, tag="bvb", name="bvb")

            # --- input DMAs ----------------------------------------------
            # sync queue order = critical path order: wk+xtkv first (K proj
            # streams per-d), wv (V proj), wq + xt[i=0] (Q proj of block 0),
            # xt[i=1], wo (needed at block 1), xt[i=2,3].
            for d in range(ND):
                nc.sync.dma_start(out=twk[d], in_=wk[d * 128:(d + 1) * 128, :])
            for d in range(ND):
                nc.sync.dma_start(out=txkv[d], in_=xtkv[d * 128:(d + 1) * 128, :])
            for d in range(ND):
                nc.sync.dma_start(out=twv[d], in_=wv[d * 128:(d + 1) * 128, :])
            for d in range(ND):
                nc.sync.dma_start(out=twq[d], in_=wq[d * 128:(d + 1) * 128, :])
            for i in (0, 1):
                for d in range(ND):
                    nc.sync.dma_start(
                        out=txt[d][i],
                        in_=xt[d * 128:(d + 1) * 128, i * 512:(i + 1) * 512],
                    )
            for c in range(NC2):
                nc.sync.dma_start(out=two[c], in_=wo[c * 128:(c + 1) * 128, :])
            for i in (2, 3):
                for d in range(ND):
                    nc.sync.dma_start(
                        out=txt[d][i],
                        in_=xt[d * 128:(d + 1) * 128, i * 512:(i + 1) * 512],
                    )
            # small tiles on the gpsimd (software DGE) queue
            for c in range(NC2):
                nc.gpsimd.dma_start(
                    out=tbq_raw[c], in_=bq[c * 128:(c + 1) * 128].unsqueeze(1)
                )
                nc.gpsimd.dma_start(
                    out=tew_raw[c], in_=ew[c * 128:(c + 1) * 128].unsqueeze(1)
                )
                nc.gpsimd.dma_start(
                    out=tbiask[c], in_=bk[c * 128:(c + 1) * 128].unsqueeze(1)
                )
                nc.vector.tensor_add(out=tbiasq[c], in0=tbq_raw[c], in1=tew_raw[c])
            for j in range(nj):
                nc.gpsimd.dma_start(
                    out=tmb[j], in_=maskb[j * 128:(j + 1) * 128].unsqueeze(1)
                )
            bvap = bv.ap()
            bv_bcast = bass.AP(
                tensor=bvap.tensor, offset=bvap.offset, ap=[[0, 128]] + list(bvap.ap)
            )
            nc.gpsimd.dma_start(out=tbvb, in_=bv_bcast)
            # memset cannot write float32r, so round ones through the DVE.
            nc.vector.memset(tonesf, 1.0)
            # rows of tstage between the 32h anchors are swept by the batched
            # reciprocal; init once so they stay finite
            nc.vector.memset(tstage, 1.0)
            for h in range(HPC):
                nc.vector.tensor_copy(
                    out=tones4[32 * h:32 * h + 1, :], in_=tonesf[0:1, :]
                )

            # --- phase 1: K, V projections + Q projection of block 0 -----
            # ones column of V (gpsimd copy keeps the DVE free)
            for h in range(HPC):
                for j in range(nj):
                    nc.gpsimd.tensor_copy(
                        out=tv[h][j][:, DH:DH + 1], in_=tonesf[:, 0:1]
                    )
            # KT: d-outer so each txkv[d] chunk is consumed as it lands
            with tc.tile_pool(name="pk", bufs=1, space="PSUM") as pk:
                kps = [
                    [pk.tile([128, 512], F32, tag=f"pk{ci}_{c}", name="pkt") for c in range(NC2)]
                    for ci in range(len(kchunks))
                ]
                for d in range(ND):
                    for ci, (jo, jw) in enumerate(kchunks):
                        for c in range(NC2):
                            nc.tensor.matmul(
                                kps[ci][c][:, 0:jw],
                                twk[d][:, c * 128:(c + 1) * 128],
                                txkv[d][:, jo:jo + jw],
                                start=(d == 0),
                                stop=(d == ND - 1),
                            )
                for ci, (jo, jw) in enumerate(kchunks):
                    for c in range(NC2):
                        nc.vector.tensor_scalar_add(
                            out=tkt[c][:, jo:jo + jw],
                            in0=kps[ci][c][:, 0:jw],
                            scalar1=tbiask[c],
                        )
            with (
                tc.tile_pool(name="pproj", bufs=2, space="PSUM") as pproj,
                tc.tile_pool(name="pv", bufs=2, space="PSUM") as pv,
            ):
                # V[j, n] accumulated over d, split per head (+bias bv)
                for j in range(nj):
                    ps = pv.tile([128, NCOL], F32, tag="pv", name="pvt")
                    for d in range(ND):
                        nc.tensor.matmul(
                            ps,
                            txkv[d][:, j * 128:(j + 1) * 128],
                            twv[d],
                            start=(d == 0),
                            stop=(d == ND - 1),
                        )
                    for h in range(HPC):
                        nc.vector.tensor_add(
                            out=tv[h][j][:, 0:DH],
                            in0=ps[:, h * DH:(h + 1) * DH],
                            in1=tbvb[:, h * DH:(h + 1) * DH],
                        )
                # Q projection of query block 0 (blocks 1..3 are interleaved
                # into phase 2 as pending units)
                for c in range(NC2):
                    ps = pproj.tile([128, 512], F32, tag="pp", name="ppt")
                    for d in range(ND):
                        nc.tensor.matmul(
                            ps,
                            twq[d][:, c * 128:(c + 1) * 128],
                            txt[d][0],
                            start=(d == 0),
                            stop=(d == ND - 1),
                        )
                    nc.vector.tensor_scalar_add(
                        out=tqt[c][:, 0:512], in0=ps, scalar1=tbiasq[c]
                    )

            # --- phase 2: attention + normalize + output projection ------
            # The j-loop is paced by the ACT-engine exp; one pending unit
            # (~0.9us of independent PE work) is popped per j iteration so
            # the PE queue never drains: output projection of block i-1 and
            # Q projection of block i+1.
            with (
                tc.tile_pool(name="pts", bufs=3) as pts,
                tc.tile_pool(name="obuf", bufs=6) as obuf,
                tc.tile_pool(name="ps2", bufs=2, space="PSUM") as ps2,
                tc.tile_pool(name="pot", bufs=2, space="PSUM") as pot,
                tc.tile_pool(name="plf", bufs=2, space="PSUM") as plf,
            ):
                pending = []
                state = {"tail": False}

                def emit_pf(i, so, n):
                    # one outproj unit: 2 accumulating matmuls -> copy -> DMA
                    sidx = i * 4 + so
                    ssl = slice(sidx * 128, (sidx + 1) * 128)
                    nsl = slice(n * 512, (n + 1) * 512)
                    pf = plf.tile([128, 512], F32, tag="plf", name="pft")
                    for c in range(NC2):
                        nc.tensor.matmul(
                            pf,
                            totn[c][:, ssl],
                            two[c][:, nsl],
                            start=(c == 0),
                            stop=(c == NC2 - 1),
                        )
                    ob = obuf.tile([128, 512], F32, tag="ob", name="obt")
                    if state["tail"] and (so + n) % 2:
                        # ACT is exp-free in the tail; split the copies so
                        # two outproj units pipeline through plf bufs=2
                        nc.scalar.copy(out=ob, in_=pf)
                    else:
                        nc.vector.tensor_copy(out=ob, in_=pf)
                    nc.sync.dma_start(out=out[ssl, nsl], in_=ob)

                def emit_norm(i):
                    # batched reciprocal + bf16 cast of 1/l, then per-head
                    # ones-row broadcast + normalize (deferred off the block
                    # boundary into later j-loop pop slots)
                    isl = slice(i * 512, (i + 1) * 512)
                    nc.vector.reciprocal_approx_fast(
                        out=trecf[:, isl], in_=tstage[:, isl]
                    )
                    nc.vector.tensor_copy(out=trecb[:, isl], in_=trecf[:, isl])
                    for c in range(NC2):
                        hA, hB = 2 * c, 2 * c + 1
                        plA = plf.tile([64, 512], F32, tag="plf", name="plA")
                        plB = plf.tile([64, 512], F32, tag="plf", name="plB")
                        nc.tensor.matmul(
                            plA,
                            tones4[32 * hA:32 * hA + 1, :],
                            trecb[32 * hA:32 * hA + 1, isl],
                            start=True,
                            stop=True,
                            tile_position=(32 * hA, 0),
                        )
                        nc.tensor.matmul(
                            plB,
                            tones4[32 * hB:32 * hB + 1, :],
                            trecb[32 * hB:32 * hB + 1, isl],
                            start=True,
                            stop=True,
                            tile_position=(32 * hB, 0),
                        )
                        nc.vector.tensor_mul(
                            out=totn[c][0:64, isl], in0=tot[c][0:64, isl], in1=plA
                        )
                        nc.vector.tensor_mul(
                            out=totn[c][64:128, isl], in0=tot[c][64:128, isl], in1=plB
                        )

                def emit_q(i, c):
                    # one Q-proj unit: 8 accumulating matmuls (borrowing a
                    # ps2 tile; scores j+1 uses the other buffer) + bias add
                    isl = slice(i * 512, (i + 1) * 512)
                    ps = ps2.tile([128, 1024], F32, tag="ps", name="pqt")
                    for d in range(ND):
                        nc.tensor.matmul(
                            ps[:, 0:512],
                            twq[d][:, c * 128:(c + 1) * 128],
                            txt[d][i],
                            start=(d == 0),
                            stop=(d == ND - 1),
                        )
                    nc.vector.tensor_scalar_add(
                        out=tqt[c][:, isl], in0=ps[:, 0:512], scalar1=tbiasq[c]
                    )

                for i in range(NI):
                    isl = slice(i * 512, (i + 1) * 512)
                    if i + 1 < NI:
                        for c in range(NC2):
                            pending.append((i + 1, lambda i=i, c=c: emit_q(i + 1, c)))
                    # force-emit units whose results this block depends on
                    # (Q projection of block i) if the queue hasn't drained
                    while any(dl <= i for dl, _ in pending):
                        pending.pop(0)[1]()
                    for c in range(NC2):
                        hA, hB = 2 * c, 2 * c + 1
                        potA = pot.tile([DH + 1, 512], F32, tag="pot", name="pott")
                        potB = pot.tile([DH + 1, 512], F32, tag="pot", name="pott")
                        pts_hist = []
                        for j in range(nj):
                            pscore = ps2.tile([128, 1024], F32, tag="ps", name="pscore")
                            nc.tensor.matmul(
                                pscore[:, 0:512],
                                tkt[c][0:64, j * 128:(j + 1) * 128],
                                tqt[c][0:64, isl],
                                start=True,
                                stop=True,
                                tile_position=(0, 0),
                            )
                            nc.tensor.matmul(
                                pscore[:, 512:1024],
                                tkt[c][64:128, j * 128:(j + 1) * 128],
                                tqt[c][64:128, isl],
                                start=True,
                                stop=True,
                                tile_position=(64, 0),
                            )
                            pt = pts.tile([128, 1024], BF16, tag="pt", name="ptile")
                            nc.scalar.activation(
                                out=pt,
                                in_=pscore,
                                func=AF.Exp,
                                bias=tmb[j],
                                scale=1.0 / math.sqrt(DH),
                            )
                            pts_hist.append(pt)
                            if j > 0:
                                pprev = pts_hist[j - 1]
                                nc.tensor.matmul(
                                    potA, tv[hA][j - 1], pprev[:, 0:512],
                                    start=(j - 1 == 0), stop=False,
                                )
                                nc.tensor.matmul(
                                    potB, tv[hB][j - 1], pprev[:, 512:1024],
                                    start=(j - 1 == 0), stop=False,
                                )
                            if pending:
                                pending.pop(0)[1]()
                        nc.tensor.matmul(
                            potA, tv[hA][nj - 1], pts_hist[nj - 1][:, 0:512],
                            start=(nj == 1), stop=True,
                        )
                        nc.tensor.matmul(
                            potB, tv[hB][nj - 1], pts_hist[nj - 1][:, 512:1024],
                            start=(nj == 1), stop=True,
                        )
                        nc.vector.tensor_copy(out=tot[c][0:64, isl], in_=potA[0:DH, :])
                        nc.vector.tensor_copy(out=tot[c][64:128, isl], in_=potB[0:DH, :])
                        nc.vector.tensor_copy(
                            out=tstage[32 * hA:32 * hA + 1, isl],
                            in_=potA[DH:DH + 1, :],
                        )
                        nc.vector.tensor_copy(
                            out=tstage[32 * hB:32 * hB + 1, isl],
                            in_=potB[DH:DH + 1, :],
                        )
                    pending.append((NI + 1, lambda i=i: emit_norm(i)))
                    for so in range(4):
                        for n in range(2):
                            pending.append(
                                (NI + 1, lambda i=i, so=so, n=n: emit_pf(i, so, n))
                            )
                state["tail"] = True
                while pending:
                    pending.pop(0)[1]()

    nc.compile()
    return nc


def _get_program(skv):
    if skv not in _PROGRAM_CACHE:
        _PROGRAM_CACHE[skv] = build_program(skv)
    return _PROGRAM_CACHE[skv]


def _shard_inputs(hidden_states, attention_mask, Wq, bq, Wk, bk, Wv, bv,
                  emotion_w, Wo, bo):
    hs = np.asarray(hidden_states, dtype=np.float32)
    mask = np.asarray(attention_mask)
    Wq = np.asarray(Wq, dtype=np.float32)
    Wk = np.asarray(Wk, dtype=np.float32)
    Wv = np.asarray(Wv, dtype=np.float32)
    Wo = np.asarray(Wo, dtype=np.float32)
    bq = np.asarray(bq, dtype=np.float32)
    bk = np.asarray(bk, dtype=np.float32)
    bv = np.asarray(bv, dtype=np.float32)
    ew = np.asarray(emotion_w, dtype=np.float32)

    idx = [np.nonzero(mask[b])[0] for b in range(B)]
    sv = max(len(ix) for ix in idx)
    skv = max(128, ((sv + 127) // 128) * 128)

    in_maps = []
    for b in range(B):
        xt_b = np.ascontiguousarray(hs[b].T.astype(BF16_NP))  # [D, S]
        xtkv_b = np.zeros((D, skv), dtype=BF16_NP)
        xtkv_b[:, : len(idx[b])] = hs[b][idx[b]].T.astype(BF16_NP)
        maskb_b = np.zeros(skv, dtype=np.float32)
        maskb_b[len(idx[b]):] = -1e30
        for hg in range(H // HPC):
            cols = slice(hg * NCOL, (hg + 1) * NCOL)
            in_maps.append(
                {
                    "xt": xt_b,
                    "xtkv": xtkv_b,
                    "wq": np.ascontiguousarray(Wq[:, cols].astype(BF16_NP)),
                    "wk": np.ascontiguousarray(Wk[:, cols].astype(BF16_NP)),
                    "wv": np.ascontiguousarray(Wv[:, cols].astype(BF16_NP)),
                    "wo": np.ascontiguousarray(Wo[cols, :].astype(BF16_NP)),
                    "bq": np.ascontiguousarray(bq[cols]),
                    "bk": np.ascontiguousarray(bk[cols]),
                    "bv": np.ascontiguousarray(bv[cols]),
                    "ew": np.ascontiguousarray(
                        ew[hg * HPC:(hg + 1) * HPC].reshape(NCOL)
                    ),
                    "maskb": maskb_b,
                }
            )
    return in_maps, skv, np.asarray(bo, dtype=np.float32)


def run(inputs, trace=False, trace_kwargs=None):
    in_maps, skv, bo = _shard_inputs(**inputs)
    nc = _get_program(skv)
    res = run_bass_kernel_spmd(
        nc,
        in_maps,
        core_ids=list(range(8)),
        trace=trace,
        **(trace_kwargs or {}),
    )
    out = np.zeros((B, S, D), dtype=np.float32)
    for b in range(B):
        acc = np.zeros((S, D), dtype=np.float64)
        for hg in range(4):
            acc += res.results[b * 4 + hg]["out"]
        out[b] = (acc + bo).astype(np.float32)
    return out, res


def kernel(**inputs):
    out, _ = run(inputs, trace=False)
    return out
